# revision 22
# baseline (speedup 1.0000x reference)
"""Boundary-map kernel for Trainium2 (Bass/Tile), 8-core SPMD.

Math: a pixel is an edge pixel iff some 4-adjacent pair of pixels with
different labels lies inside its radius-2 Euclidean disk (on the 2-zero-padded
label map; verified exhaustively against the reference).  With
    XH(i,j) = x(i,j) ^ x(i,j+1)      (horizontal pair diffs)
    XV(i,j) = x(i,j) ^ x(i+1,j)      (vertical pair diffs)
    edge(p) = [ sum_{s in SH} XH(p+s) + sum_{s in SV} XV(p+s) ] > 0
    SH = {(0,-2),(0,-1),(0,0),(0,1),(+-1,-1),(+-1,0)}
    SV = {(-2,0),(-1,0),(0,0),(1,0),(-1,+-1),(0,+-1)}

Layout: rows in partitions; columns byte-PACKED 2-per-int16 lane (lo byte =
left image half, hi byte = right half, each half with its own 2-col halo).
DVE computes XH/XV (packed int16 xor, 2x mode) plus two fused column-pair
sums u = XH(0,-1)+XH(0,0) and s = XV(0,-1)+XV(0,+1) (carry-free: bytes<64).
All row mixing runs on the TensorEngine as fp8 band matmuls over the int8
byte view reinterpreted as float8e4: every byte is < 128 so it reads as a
NONNEGATIVE fp8 value that is zero iff the byte is zero; sums of such values
are positive iff any contributing byte is nonzero, which is all the > 0 test
needs (verified on HW incl. the denormal byte range 1..7).  Per 512-byte
PSUM chunk, 5 accumulating matmuls cover both dilation sets:
    w_v3.u8[b]  ->  SH terms (0,-1),(0,0),(+-1,-1),(+-1,0)
    I.XH8[b-4]  ->  SH (0,-2)        I.XH8[b+2]  ->  SH (0,+1)
    w_v4.XV8[b] ->  SV (-2..1, 0)    w_v2.s8[b]  ->  SV (-1,+-1),(0,+-1)
ScalarE extracts Sign(PSUM) -> int8 0/1, DMAed out (host just casts).
No SBUF->SBUF DMA anywhere (it measured ~20-30 GB/s — pathologically slow);
the only DMAs are parallel HBM loads, the weight load, and outputs.

Sharding: 2 batches x 4 col-quarters -> 8 cores.  Each core: two 125-out-row
bands (full width, tiles [128p, 1028f]) + one 24-row x 512-col strip
([28p, 260f]) covering the last 24 rows of its batch.
"""

import numpy as np
import ml_dtypes
from contextlib import ExitStack

import concourse.bass as bass
import concourse.bacc as bacc
import concourse.mybir as mybir
import concourse.tile as tile
from concourse import bass_utils

I16 = mybir.dt.int16
I8 = mybir.dt.int8
F32 = mybir.dt.float32
FP8 = mybir.dt.float8e4
OP = mybir.AluOpType
AF = mybir.ActivationFunctionType

B, H, W = 2, 1024, 2048
NCORES = 8
BAND = 125           # output rows per main band
NBAND = 8            # bands per batch
F = 1028             # packed free width for main bands (2 planes of 1028 cols)
SROWS = H - NBAND * BAND   # 24 strip rows per batch
SF = 260             # strip packed free width (2 planes of 260)
CHUNK = 512          # PSUM bank width in fp32

PROFILE = False
LAST_EXEC_NS = None
LAST_RESULTS = None

WNAMES = ("w_v3", "w_v4", "w_v2", "w_i")


def _band(taps, P=128):
    w = np.zeros((P, P), np.float32)  # [k, m]: out row m sums w[k,m]*src[k]
    for m in range(P):
        for t in taps:
            k = m + t
            if 0 <= k < P:
                w[k, m] = 1.0
    return w.astype(ml_dtypes.float8_e4m3fn)


def make_weights():
    wd = {
        "w_v3": _band([-1, 0, 1]),
        "w_v4": _band([-2, -1, 0, 1]),
        "w_v2": _band([-1, 0]),
        "w_i": _band([0]),
    }
    return np.concatenate([wd[k] for k in WNAMES], axis=1)


class Jb:
    def __init__(self, sb, ps, src, P, C, dst, V, tg):
        self.src, self.P, self.C, self.dst, self.V = src, P, C, dst, V
        for nm in ("x", "xp", "XH", "XV", "u", "s", "XH2"):
            setattr(self, nm, sb.tile([P, C], I16, name=f"{nm}{tg}",
                                      tag=f"{nm}{tg}"))
        self.e8 = sb.tile([P, 2 * C], I8, name=f"e8{tg}", tag=f"e8{tg}")
        self.ps = ps
        self.tg = tg


def _diffs(nc, j, half):
    """Column-half-sliced diff stage: half 0 covers lanes [0, C//2+3),
    half 1 covers [C//2-3, C) (writes split disjointly at C//2), so the PE
    can start consuming half 0 while half 1 is still being produced.
    half=None runs the full width (strip)."""
    P, C = j.P, j.C
    M = C // 2
    if half is None:
        xh_r = (0, C - 1); xv_r = (0, C); u_r = (1, C); x2_r = (2, C - 1); s_r = (1, C - 1)
    elif half == 0:
        xh_r = (0, M + 3); xv_r = (0, M + 3); u_r = (1, M); x2_r = (2, M); s_r = (1, M)
    else:
        xh_r = (M + 3, C - 1); xv_r = (M + 3, C); u_r = (M, C); x2_r = (M, C - 1); s_r = (M, C - 1)
    def tt(dst, a, b, op, lo, hi, da, db):
        nc.vector.tensor_tensor(out=dst[:, lo:hi], in0=a[:, lo + da:hi + da],
                                in1=b[:, lo + db:hi + db], op=op)
    tt(j.XH, j.x, j.x, OP.bitwise_xor, *xh_r, 0, 1)
    tt(j.XV, j.x, j.xp, OP.bitwise_xor, *xv_r, 0, 0)
    tt(j.u, j.XH, j.XH, OP.add, *u_r, -1, 0)
    # XH2 = XH(0,-2) + XH(0,+1): merges the two identity-weight taps into
    # one matmul operand (4 matmuls per chunk instead of 5)
    tt(j.XH2, j.XH, j.XH, OP.add, *x2_r, -2, 1)
    tt(j.s, j.XV, j.XV, OP.add, *s_r, -1, 1)


def _pe_stage(nc, wt, j):
    P, C = j.P, j.C
    u8 = j.u[:, :].bitcast(FP8)
    s8 = j.s[:, :].bitcast(FP8)
    XH28 = j.XH2[:, :].bitcast(FP8)
    XV8 = j.XV[:, :].bitcast(FP8)
    nout = 2 * C - 8  # valid out bytes 4 .. 2C-5
    for c0 in range(0, nout, CHUNK):
        n = min(CHUNK, nout - c0)
        b0 = 4 + c0
        # per-chunk PSUM tiles (bufs=8 rotation) so chunk c+1's matmuls never
        # wait on chunk c's Sign readback (tile-granular dependency tracking)
        pt = j.ps.tile([128, CHUNK], F32, name=f"pe{j.tg}c{c0}", tag="pe")
        pc = pt[0:P, 0:n]
        nc.tensor.matmul(out=pc, lhsT=wt["w_v4"][0:P, 0:P], rhs=XV8[:, b0:b0 + n],
                         start=True, stop=False)
        nc.tensor.matmul(out=pc, lhsT=wt["w_v3"][0:P, 0:P], rhs=u8[:, b0:b0 + n],
                         start=False, stop=False)
        nc.tensor.matmul(out=pc, lhsT=wt["w_i"][0:P, 0:P], rhs=XH28[:, b0:b0 + n],
                         start=False, stop=False)
        nc.tensor.matmul(out=pc, lhsT=wt["w_v2"][0:P, 0:P], rhs=s8[:, b0:b0 + n],
                         start=False, stop=True)
        nc.scalar.activation(out=j.e8[:, b0:b0 + n], in_=pc, func=AF.Sign)


def _extract(nc, j):
    nout = 2 * j.C - 8
    nc.sync.dma_start(j.dst, j.e8[2:2 + j.V, 4:4 + nout])


def build_nc():
    # Bacc: its compile() legalizes multi-wait instructions via
    # generate_event_semaphores (the TileContext tail drain needs it).
    nc = bacc.Bacc("TRN2", target_bir_lowering=False, debug=False)
    s0 = nc.dram_tensor("s0", [130, F], I16, kind="ExternalInput").ap()
    s1 = nc.dram_tensor("s1", [130, F], I16, kind="ExternalInput").ap()
    ss = nc.dram_tensor("ss", [SROWS + 6, SF], I16, kind="ExternalInput").ap()
    wcat = nc.dram_tensor("wcat", [128, 128 * len(WNAMES)], FP8,
                          kind="ExternalInput").ap()
    y0 = nc.dram_tensor("y0", [BAND, 2 * F - 8], I8, kind="ExternalOutput").ap()
    y1 = nc.dram_tensor("y1", [BAND, 2 * F - 8], I8, kind="ExternalOutput").ap()
    ys = nc.dram_tensor("ys", [SROWS, 2 * SF - 8], I8, kind="ExternalOutput").ap()

    with ExitStack() as ctx:
        tc = ctx.enter_context(tile.TileContext(nc))
        wp = ctx.enter_context(tc.tile_pool(name="wp", bufs=1))
        sb = ctx.enter_context(tc.tile_pool(name="sb", bufs=1))
        ps = ctx.enter_context(tc.tile_pool(name="ps", bufs=6, space="PSUM"))
        pw = ctx.enter_context(tc.tile_pool(name="pw", bufs=1, space="PSUM"))
        wtile = wp.tile([128, 128 * len(WNAMES)], FP8, name="wtile")
        wt = {k: wtile[:, 128 * i:128 * (i + 1)] for i, k in enumerate(WNAMES)}
        jobs = [
            Jb(sb, ps, s0, 128, F, y0, BAND, "0"),
            Jb(sb, ps, s1, 128, F, y1, BAND, "1"),
            Jb(sb, ps, ss, SROWS + 4, SF, ys, SROWS, "s"),
        ]
        for j in jobs:
            nc.sync.dma_start(j.x[:, :], j.src[1:j.P + 1, :])
        nc.sync.dma_start(wtile[:, :], wcat)
        for j in jobs:
            nc.scalar.dma_start(j.xp[:, :], j.src[2:j.P + 2, :])
        # PE pstate warmup: ~10 dummy matmuls on a zeroed tile, issued while
        # the input loads are still in flight, so the real matmul stream
        # starts at the full 2.4 GHz clock instead of paying the ramp.
        warm = sb.tile([128, CHUNK], FP8, name="warm", tag="warm")
        nc.gpsimd.memset(warm[:, :], 0)
        pwt = pw.tile([128, CHUNK], F32, name="pwt", tag="pwt")
        for _ in range(10):
            nc.tensor.matmul(out=pwt[:, :], lhsT=warm[:, 0:128], rhs=warm[:, :],
                             start=True, stop=True)
        _diffs(nc, jobs[0], 0)
        _diffs(nc, jobs[0], 1)
        _diffs(nc, jobs[1], 0)
        _diffs(nc, jobs[1], 1)
        _diffs(nc, jobs[2], None)
        for j in jobs:
            _pe_stage(nc, wt, j)
            _extract(nc, j)
    nc.compile()
    return nc


def make_in_maps(gtmasks):
    lab = np.asarray(gtmasks)[:, 0].astype(np.uint8)  # labels 0..19 fit a byte
    wcat = make_weights()
    packed = []
    raw = []
    for b in range(B):
        A = np.pad(lab[b], 2)  # [H+4, W+4] = [1028, 2052]
        # clamp rows on both ends: row i of A2 = padded row i-1, rows -1 and
        # 1028 duplicated (their values only reach non-output partitions)
        A2 = np.vstack([A[0:1], A, A[-1:]])  # [1030, 2052]
        P = (A2[:, 0:F].astype(np.uint16)
             | (A2[:, W // 2:W // 2 + F].astype(np.uint16) << 8)).view(np.int16)
        packed.append(P)
        raw.append(A2)
    in_maps = []
    for c in range(NCORES):
        b, qq = divmod(c, 4)
        A2 = raw[b]
        r0 = NBAND * BAND  # first strip out-row
        c0 = 512 * qq
        slo = A2[r0:r0 + SROWS + 6, c0:c0 + SF]
        shi = A2[r0:r0 + SROWS + 6, c0 + SF - 4:c0 + 2 * SF - 4]
        sp = (slo.astype(np.uint16) | (shi.astype(np.uint16) << 8)).view(np.int16)
        k0, k1 = 2 * qq, 2 * qq + 1
        im = {
            "s0": np.ascontiguousarray(packed[b][BAND * k0:BAND * k0 + 130, :]),
            "s1": np.ascontiguousarray(packed[b][BAND * k1:BAND * k1 + 130, :]),
            "ss": np.ascontiguousarray(sp),
            "wcat": wcat,
        }
        in_maps.append(im)
    return in_maps


def assemble(results):
    out = np.zeros((B, 1, H, W), np.int32)
    for c in range(NCORES):
        b, qq = divmod(c, 4)
        for j, k in enumerate((2 * qq, 2 * qq + 1)):
            v = results[c][f"y{j}"].reshape(BAND, F - 4, 2)
            rows = slice(BAND * k, BAND * (k + 1))
            out[b, 0, rows, 0:W // 2] = v[:, :, 0] != 0
            out[b, 0, rows, W // 2:W] = v[:, :, 1] != 0
        vs = results[c]["ys"].reshape(SROWS, SF - 4, 2)
        c0 = 512 * qq
        out[b, 0, NBAND * BAND:H, c0:c0 + 256] = vs[:, :, 0] != 0
        out[b, 0, NBAND * BAND:H, c0 + 256:c0 + 512] = vs[:, :, 1] != 0
    return out


def kernel(gtmasks):
    global LAST_EXEC_NS, LAST_RESULTS
    in_maps = make_in_maps(gtmasks)
    nc = build_nc()
    res = bass_utils.run_bass_kernel_spmd(
        nc, in_maps, core_ids=list(range(NCORES)), trace=PROFILE)
    LAST_EXEC_NS = res.exec_time_ns
    LAST_RESULTS = res
    return assemble(res.results)


# revision 23
# speedup vs baseline: 1.1488x; 1.1488x over previous
"""Boundary-map kernel for Trainium2 (Bass/Tile), 8-core SPMD.

Math: a pixel is an edge pixel iff some 4-adjacent pair of pixels with
different labels lies inside its radius-2 Euclidean disk (on the 2-zero-padded
label map; verified exhaustively against the reference).  With
    XH(i,j) = x(i,j) ^ x(i,j+1)      (horizontal pair diffs)
    XV(i,j) = x(i,j) ^ x(i+1,j)      (vertical pair diffs)
    edge(p) = [ sum_{s in SH} XH(p+s) + sum_{s in SV} XV(p+s) ] > 0
    SH = {(0,-2),(0,-1),(0,0),(0,1),(+-1,-1),(+-1,0)}
    SV = {(-2,0),(-1,0),(0,0),(1,0),(-1,+-1),(0,+-1)}

Layout: rows in partitions; columns byte-PACKED 2-per-int16 lane (lo byte =
left image half, hi byte = right half, each half with its own 2-col halo).
DVE computes XH/XV (packed int16 xor, 2x mode) plus two fused column-pair
sums u = XH(0,-1)+XH(0,0) and s = XV(0,-1)+XV(0,+1) (carry-free: bytes<64).
All row mixing runs on the TensorEngine as fp8 band matmuls over the int8
byte view reinterpreted as float8e4: every byte is < 128 so it reads as a
NONNEGATIVE fp8 value that is zero iff the byte is zero; sums of such values
are positive iff any contributing byte is nonzero, which is all the > 0 test
needs (verified on HW incl. the denormal byte range 1..7).  Per 512-byte
PSUM chunk, 5 accumulating matmuls cover both dilation sets:
    w_v3.u8[b]  ->  SH terms (0,-1),(0,0),(+-1,-1),(+-1,0)
    I.XH8[b-4]  ->  SH (0,-2)        I.XH8[b+2]  ->  SH (0,+1)
    w_v4.XV8[b] ->  SV (-2..1, 0)    w_v2.s8[b]  ->  SV (-1,+-1),(0,+-1)
ScalarE extracts Sign(PSUM) -> int8 0/1, DMAed out (host just casts).
No SBUF->SBUF DMA anywhere (it measured ~20-30 GB/s — pathologically slow);
the only DMAs are parallel HBM loads, the weight load, and outputs.

Sharding: 2 batches x 4 col-quarters -> 8 cores.  Each core: two 125-out-row
bands (full width, tiles [128p, 1028f]) + one 24-row x 512-col strip
([28p, 260f]) covering the last 24 rows of its batch.
"""

import numpy as np
import ml_dtypes
from contextlib import ExitStack

import concourse.bass as bass
import concourse.bacc as bacc
import concourse.mybir as mybir
import concourse.tile as tile
from concourse import bass_utils

I16 = mybir.dt.int16
I8 = mybir.dt.int8
F32 = mybir.dt.float32
FP8 = mybir.dt.float8e4
OP = mybir.AluOpType
AF = mybir.ActivationFunctionType

B, H, W = 2, 1024, 2048
NCORES = 8
BAND = 125           # output rows per main band
NBAND = 8            # bands per batch
F = 1028             # packed free width for main bands (2 planes of 1028 cols)
SROWS = H - NBAND * BAND   # 24 strip rows per batch
SF = 260             # strip packed free width (2 planes of 260)
CHUNK = 512          # PSUM bank width in fp32

PROFILE = False
LAST_EXEC_NS = None
LAST_RESULTS = None

WNAMES = ("w_v3", "w_v4", "w_v2", "w_i")


def _band(taps, P=128):
    w = np.zeros((P, P), np.float32)  # [k, m]: out row m sums w[k,m]*src[k]
    for m in range(P):
        for t in taps:
            k = m + t
            if 0 <= k < P:
                w[k, m] = 1.0
    return w.astype(ml_dtypes.float8_e4m3fn)


def make_weights():
    wd = {
        "w_v3": _band([-1, 0, 1]),
        "w_v4": _band([-2, -1, 0, 1]),
        "w_v2": _band([-1, 0]),
        "w_i": _band([0]),
    }
    return np.concatenate([wd[k] for k in WNAMES], axis=1)


class Jb:
    def __init__(self, sb, ps, src, P, C, dst, V, tg):
        self.src, self.P, self.C, self.dst, self.V = src, P, C, dst, V
        for nm in ("x", "xp", "XH", "XV", "u", "s", "XH2"):
            setattr(self, nm, sb.tile([P, C], I16, name=f"{nm}{tg}",
                                      tag=f"{nm}{tg}"))
        self.e8 = sb.tile([P, 2 * C], I8, name=f"e8{tg}", tag=f"e8{tg}")
        self.ps = ps
        self.tg = tg


def _diffs(nc, j, half):
    """Column-half-sliced diff stage: half 0 covers lanes [0, C//2+3),
    half 1 covers [C//2-3, C) (writes split disjointly at C//2), so the PE
    can start consuming half 0 while half 1 is still being produced.
    half=None runs the full width (strip)."""
    P, C = j.P, j.C
    M = C // 2
    if half is None:
        xh_r = (0, C - 1); xv_r = (0, C); u_r = (1, C); x2_r = (2, C - 1); s_r = (1, C - 1)
    elif half == 0:
        xh_r = (0, M + 3); xv_r = (0, M + 3); u_r = (1, M); x2_r = (2, M); s_r = (1, M)
    else:
        xh_r = (M + 3, C - 1); xv_r = (M + 3, C); u_r = (M, C); x2_r = (M, C - 1); s_r = (M, C - 1)
    def tt(dst, a, b, op, lo, hi, da, db):
        nc.vector.tensor_tensor(out=dst[:, lo:hi], in0=a[:, lo + da:hi + da],
                                in1=b[:, lo + db:hi + db], op=op)
    tt(j.XH, j.x, j.x, OP.bitwise_xor, *xh_r, 0, 1)
    tt(j.XV, j.x, j.xp, OP.bitwise_xor, *xv_r, 0, 0)
    tt(j.u, j.XH, j.XH, OP.add, *u_r, -1, 0)
    # XH2 = XH(0,-2) + XH(0,+1): merges the two identity-weight taps into
    # one matmul operand (4 matmuls per chunk instead of 5)
    tt(j.XH2, j.XH, j.XH, OP.add, *x2_r, -2, 1)
    tt(j.s, j.XV, j.XV, OP.add, *s_r, -1, 1)


def _pe_stage(nc, wt, j):
    P, C = j.P, j.C
    u8 = j.u[:, :].bitcast(FP8)
    s8 = j.s[:, :].bitcast(FP8)
    XH28 = j.XH2[:, :].bitcast(FP8)
    XV8 = j.XV[:, :].bitcast(FP8)
    nout = 2 * C - 8  # valid out bytes 4 .. 2C-5
    for c0 in range(0, nout, CHUNK):
        n = min(CHUNK, nout - c0)
        b0 = 4 + c0
        # per-chunk PSUM tiles (bufs=8 rotation) so chunk c+1's matmuls never
        # wait on chunk c's Sign readback (tile-granular dependency tracking)
        pt = j.ps.tile([128, CHUNK], F32, name=f"pe{j.tg}c{c0}", tag="pe")
        pc = pt[0:P, 0:n]
        nc.tensor.matmul(out=pc, lhsT=wt["w_v4"][0:P, 0:P], rhs=XV8[:, b0:b0 + n],
                         start=True, stop=False)
        nc.tensor.matmul(out=pc, lhsT=wt["w_v3"][0:P, 0:P], rhs=u8[:, b0:b0 + n],
                         start=False, stop=False)
        nc.tensor.matmul(out=pc, lhsT=wt["w_i"][0:P, 0:P], rhs=XH28[:, b0:b0 + n],
                         start=False, stop=False)
        nc.tensor.matmul(out=pc, lhsT=wt["w_v2"][0:P, 0:P], rhs=s8[:, b0:b0 + n],
                         start=False, stop=True)
        nc.scalar.activation(out=j.e8[:, b0:b0 + n], in_=pc, func=AF.Sign)
        nc.sync.dma_start(j.dst[:, c0:c0 + n], j.e8[2:2 + j.V, b0:b0 + n])


def _extract(nc, j):
    pass


def build_nc():
    # Bacc: its compile() legalizes multi-wait instructions via
    # generate_event_semaphores (the TileContext tail drain needs it).
    nc = bacc.Bacc("TRN2", target_bir_lowering=False, debug=False)
    s0 = nc.dram_tensor("s0", [130, F], I16, kind="ExternalInput").ap()
    s1 = nc.dram_tensor("s1", [130, F], I16, kind="ExternalInput").ap()
    ss = nc.dram_tensor("ss", [SROWS + 6, SF], I16, kind="ExternalInput").ap()
    wcat = nc.dram_tensor("wcat", [128, 128 * len(WNAMES)], FP8,
                          kind="ExternalInput").ap()
    y0 = nc.dram_tensor("y0", [BAND, 2 * F - 8], I8, kind="ExternalOutput").ap()
    y1 = nc.dram_tensor("y1", [BAND, 2 * F - 8], I8, kind="ExternalOutput").ap()
    ys = nc.dram_tensor("ys", [SROWS, 2 * SF - 8], I8, kind="ExternalOutput").ap()

    with ExitStack() as ctx:
        tc = ctx.enter_context(tile.TileContext(nc))
        wp = ctx.enter_context(tc.tile_pool(name="wp", bufs=1))
        sb = ctx.enter_context(tc.tile_pool(name="sb", bufs=1))
        ps = ctx.enter_context(tc.tile_pool(name="ps", bufs=6, space="PSUM"))
        pw = ctx.enter_context(tc.tile_pool(name="pw", bufs=1, space="PSUM"))
        wtile = wp.tile([128, 128 * len(WNAMES)], FP8, name="wtile")
        wt = {k: wtile[:, 128 * i:128 * (i + 1)] for i, k in enumerate(WNAMES)}
        jobs = [
            Jb(sb, ps, s0, 128, F, y0, BAND, "0"),
            Jb(sb, ps, s1, 128, F, y1, BAND, "1"),
            Jb(sb, ps, ss, SROWS + 4, SF, ys, SROWS, "s"),
        ]
        for j in jobs:
            nc.sync.dma_start(j.x[:, :], j.src[1:j.P + 1, :])
        nc.sync.dma_start(wtile[:, :], wcat)
        for j in jobs:
            nc.scalar.dma_start(j.xp[:, :], j.src[2:j.P + 2, :])
        # PE pstate warmup: ~10 dummy matmuls on a zeroed tile, issued while
        # the input loads are still in flight, so the real matmul stream
        # starts at the full 2.4 GHz clock instead of paying the ramp.
        warm = sb.tile([128, CHUNK], FP8, name="warm", tag="warm")
        nc.gpsimd.memset(warm[:, :], 0)
        pwt = pw.tile([128, CHUNK], F32, name="pwt", tag="pwt")
        for _ in range(10):
            nc.tensor.matmul(out=pwt[:, :], lhsT=warm[:, 0:128], rhs=warm[:, :],
                             start=True, stop=True)
        _diffs(nc, jobs[0], 0)
        _diffs(nc, jobs[0], 1)
        _diffs(nc, jobs[1], 0)
        _diffs(nc, jobs[1], 1)
        _diffs(nc, jobs[2], None)
        for j in jobs:
            _pe_stage(nc, wt, j)
            _extract(nc, j)
    nc.compile()
    return nc


def make_in_maps(gtmasks):
    lab = np.asarray(gtmasks)[:, 0].astype(np.uint8)  # labels 0..19 fit a byte
    wcat = make_weights()
    packed = []
    raw = []
    for b in range(B):
        A = np.pad(lab[b], 2)  # [H+4, W+4] = [1028, 2052]
        # clamp rows on both ends: row i of A2 = padded row i-1, rows -1 and
        # 1028 duplicated (their values only reach non-output partitions)
        A2 = np.vstack([A[0:1], A, A[-1:]])  # [1030, 2052]
        P = (A2[:, 0:F].astype(np.uint16)
             | (A2[:, W // 2:W // 2 + F].astype(np.uint16) << 8)).view(np.int16)
        packed.append(P)
        raw.append(A2)
    in_maps = []
    for c in range(NCORES):
        b, qq = divmod(c, 4)
        A2 = raw[b]
        r0 = NBAND * BAND  # first strip out-row
        c0 = 512 * qq
        slo = A2[r0:r0 + SROWS + 6, c0:c0 + SF]
        shi = A2[r0:r0 + SROWS + 6, c0 + SF - 4:c0 + 2 * SF - 4]
        sp = (slo.astype(np.uint16) | (shi.astype(np.uint16) << 8)).view(np.int16)
        k0, k1 = 2 * qq, 2 * qq + 1
        im = {
            "s0": np.ascontiguousarray(packed[b][BAND * k0:BAND * k0 + 130, :]),
            "s1": np.ascontiguousarray(packed[b][BAND * k1:BAND * k1 + 130, :]),
            "ss": np.ascontiguousarray(sp),
            "wcat": wcat,
        }
        in_maps.append(im)
    return in_maps


def assemble(results):
    out = np.zeros((B, 1, H, W), np.int32)
    for c in range(NCORES):
        b, qq = divmod(c, 4)
        for j, k in enumerate((2 * qq, 2 * qq + 1)):
            v = results[c][f"y{j}"].reshape(BAND, F - 4, 2)
            rows = slice(BAND * k, BAND * (k + 1))
            out[b, 0, rows, 0:W // 2] = v[:, :, 0] != 0
            out[b, 0, rows, W // 2:W] = v[:, :, 1] != 0
        vs = results[c]["ys"].reshape(SROWS, SF - 4, 2)
        c0 = 512 * qq
        out[b, 0, NBAND * BAND:H, c0:c0 + 256] = vs[:, :, 0] != 0
        out[b, 0, NBAND * BAND:H, c0 + 256:c0 + 512] = vs[:, :, 1] != 0
    return out


def kernel(gtmasks):
    global LAST_EXEC_NS, LAST_RESULTS
    in_maps = make_in_maps(gtmasks)
    nc = build_nc()
    res = bass_utils.run_bass_kernel_spmd(
        nc, in_maps, core_ids=list(range(NCORES)), trace=PROFILE)
    LAST_EXEC_NS = res.exec_time_ns
    LAST_RESULTS = res
    return assemble(res.results)
